# revision 1
# baseline (speedup 1.0000x reference)
"""Trainium2 Bass kernel for nn_DenseGNOBlock (B=4, N=8192, C=64).

Reference computes, per batch b:
    q = x Wq^T + bq ; k = x Wk^T + bk ; v = x Wv^T + bv
    kernel = q k^T / sqrt(C) ; integral = kernel v / N
    out = gelu(x Ww^T + bw + integral)

No softmax, so the N x N kernel reassociates away completely. With
Gt = [x|1]^T [x|1] (65 x 65, per batch) and Wt* = [W* | b*]:
    S = k^T v = Wtk Gt Wtv^T
    out = gelu(x @ Mmat + c^T)        broadcast row bias c
    Mmat = Ww^T + a Wq^T S ;  c = bw + a S^T bq ;  a = 1/(sqrt(C) N)

Per core: one pass of x through the PE for Gt, a tiny matrix chain,
one pass for the output. Everything is exact fp32. The rest is layout
engineering:

- All weights/biases/identities/selectors ship as ONE packed
  [128, 386] input (host-prepared, transposed and ALPHA-folded where
  needed) -> a single DMA instead of a dozen small serialized ones.
  I128 doubles as e_top=[I;0], e_bot=[0;I], I64, and the row
  shift/selector matrices used to assemble the Mt variants.
- x rows are packed in "pair blocks" [1 | x_even | x_odd | 1] (130
  cols) so the x DMA lands 512B-contiguous per partition AND both
  matmuls of a column-packed G-pair get an augmented rhs for free
  (even: rhs=[1|x_e] -> [m_e|G_e], odd: rhs=[x_o|1] -> [G_o|m_o]),
  AND the forward transposes get augmented inputs for free
  ([1|x_e]^T has the ones row at row 0, [x_o|1]^T at row 64).
- Input DMAs alternate between the two HWDGE rings (SP via nc.sync,
  ACT via nc.scalar) so issue/transfer overhead pipelines.
- G/m accumulate column-packed (tile_position (0,0)/(0,64)) into
  separate PSUM banks (start=True clears has_written bank-wide), then
  fold top+bottom with matmuls against I128's column halves.
- The chain is host-shortened: T1 = Gt Wtv^T, then one matmul against
  the host-folded utq = [(a Wq^T Wtk)^T | a Wtk^T bq] plus one add of
  [Ww^T; bw] yields Mt_odd = [Mmat; c_row] directly; Mt_even =
  [c_row; Mmat] is one cyclic-row-shift matmul away.
- Forward transposes interleave with the G phase (they do not depend
  on the chain); final matmuls are K=65 augmented (bias included), so
  gelu reads PSUM and writes the output buffer directly -- no bias
  add, no transpose-back, no extra copies.

Sharding: 8 cores, core c -> batch b = c//2, half h = c%2. Each core
receives the full x_b (rotated so its own 4096 rows come first),
computes Gt over all of x_b (order-invariant), and writes its own half.
"""

import sys

for _p in ("/opt/trn_rl_repo", "/root/.axon_site/_ro/trn_rl_repo"):
    if _p not in sys.path:
        sys.path.append(_p)

import numpy as np
from contextlib import ExitStack

import concourse.bass as bass
import concourse.bacc as bacc
import concourse.mybir as mybir
import concourse.tile as tile
from concourse.bass_utils import run_bass_kernel_spmd

FP = mybir.dt.float32
FPR = mybir.dt.float32r
AF = mybir.ActivationFunctionType
MUL = mybir.AluOpType.mult
ADD = mybir.AluOpType.add

B, N, C = 4, 8192, 64
P = 128              # partitions
W = C + 1            # augmented width
NPAIR = N // (2 * P)  # 32 pair blocks per batch
BLK = 2 * C + 2      # 130 cols: [1 | x_even | x_odd | 1]
HPAIR = NPAIR // 2   # 16 own pair blocks
NCORES = 8
ALPHA = 1.0 / (np.sqrt(np.float32(C)) * np.float32(N))
DMA_GP = 2           # pair blocks per x dma_start (16 groups)
# packed weight layout (free offsets)
WPK_VT = 0           # [0:65, 0:64]    [Wv^T ; bv^T]
WPK_UTQ = 64         # [0:65, 64:129]  [(a Wq^T Wtk)^T | a Wtk^T bq]
WPK_WB = 129         # [0:65, 129:193] [Ww^T ; bw-row]
WPK_CYC = 193        # [0:65, 193:258] cyc[k,i] = (i == (k+1) mod 65)
WPK_ID = 258         # [:, 258:386]    I128
WPK_F = WPK_ID + P   # 386 total


def build_nc(act: str = "gelu") -> bass.Bass:
    act_fn = {"gelu": AF.Gelu, "identity": AF.Identity}[act]
    nc = bacc.Bacc("TRN2", target_bir_lowering=False, debug=False)

    x_d = nc.declare_dram_parameter("xb", [P, NPAIR * BLK], FP, isOutput=False)
    wpk_d = nc.declare_dram_parameter("wpk", [P, WPK_F], FP, isOutput=False)
    out_d = nc.declare_dram_parameter("out", [N // 2, C], FP, isOutput=True)

    with ExitStack() as ctx:
        tc = ctx.enter_context(tile.TileContext(nc))
        const = ctx.enter_context(tc.tile_pool(name="const", bufs=1))
        ps_g = ctx.enter_context(tc.tile_pool(name="ps_g", bufs=2, space="PSUM"))
        ps_big = ctx.enter_context(tc.tile_pool(name="ps_big", bufs=2, space="PSUM"))
        sb_t = ctx.enter_context(tc.tile_pool(name="sb_t", bufs=3))

        wpk = const.tile([P, WPK_F], FP)
        wvta = wpk[0:W, WPK_VT : WPK_VT + C]
        utq = wpk[0:W, WPK_UTQ : WPK_UTQ + W]
        wwbw = wpk[0:W, WPK_WB : WPK_WB + C]
        cyc = wpk[0:W, WPK_CYC : WPK_CYC + W]
        ident = wpk[:, WPK_ID : WPK_ID + P]
        e_top = ident[:, 0:C]            # [I64; 0]
        e_bot = ident[:, C:P]            # [0; I64]
        id64 = ident[0:C, 0:C]

        # --- stream x in: host-prebuilt pair blocks [1 | x_e | x_o | 1] ---
        # row(p, pair, j) = pair*256 + 2p + j; ones columns included by the
        # host so the transfers are fully contiguous per partition and the
        # early G matmuls wait on exactly one DMA lane; dual HWDGE rings
        xsb = const.tile([P, NPAIR, BLK], FP)
        xr = x_d[:].rearrange("p (b k) -> p b k", k=BLK)
        # a tiny pair-0 DMA goes absolutely first: every DMA pays a fixed
        # completion-receipt latency before its semaphore fires, so the
        # first PE matmul is gated by (first dma end + receipt) -- keep
        # that transfer as small as possible. Early groups ride the SP
        # ring (the ACT ring opens with a ~1.3us activation-table load).
        nc.sync.dma_start(out=xsb[:, 0:1, :], in_=xr[:, 0:1, :])
        nc.sync.dma_start(out=xsb[:, 1:2, :], in_=xr[:, 1:2, :])
        nc.sync.dma_start(out=wpk[:], in_=wpk_d[:])
        for g in range(1, NPAIR // DMA_GP):
            eng = nc.sync if g < 10 else nc.scalar
            eng.dma_start(
                out=xsb[:, g * DMA_GP : (g + 1) * DMA_GP, :],
                in_=xr[:, g * DMA_GP : (g + 1) * DMA_GP, :],
            )

        # --- PE warm-up: data-independent dummy matmuls fill the DMA
        # completion-receipt dead window so the PE clock (HAM p-state)
        # reaches full rate before real work arrives; their results are
        # never used
        warm = const.tile([P, C], FP)
        nc.vector.memset(warm[:], 1.0)
        wps = ps_big.tile([C, C], FP, tag="pt", bufs=3)
        for _ in range(3):
            nc.tensor.matmul(wps[:], warm[:], warm[:])
        nc.vector.tensor_copy(warm[0:C, :], wps[:])  # keep the tile "read"

        # --- [G|m] accumulation, column-packed ----------------------------
        # Distinct PSUM banks per group: start=True clears has_written
        # bank-wide, so interleaved groups must not share a bank.
        # gm accumulators share PSUM slots with the final-phase po tiles
        # (disjoint lifetimes), freeing banks for triple buffering
        gm_a = ps_big.tile([P, W], FP, tag="po", bufs=3)
        gm_b = ps_big.tile([P, W], FP, tag="po", bufs=3)
        # forward transposes of the own half interleave with G
        # accumulation (they only need x, not the chain)
        xt8s = []
        pt = None
        for b in range(NPAIR):
            st, sp = b == 0, b == NPAIR - 1
            # even: rows 0-63 = [m_e | G_e]
            nc.tensor.matmul(
                gm_a[0:C, :], xsb[:, b, 1 : 1 + C], xsb[:, b, 0:W],
                start=st, stop=sp, tile_position=(0, 0),
            )
            # odd: rows 64-127 = [G_o | m_o]
            nc.tensor.matmul(
                gm_b[C:P, :], xsb[:, b, 1 + C : 1 + 2 * C],
                xsb[:, b, 1 + C : BLK],
                start=st, stop=sp, tile_position=(0, C),
            )
            if b < HPAIR:
                # two augmented transposes per pair: [1|x_e] -> ones row 0,
                # [x_o|1] -> ones row 64; 2 pairs batch into one PSUM bank
                j2 = b % 2
                if j2 == 0:
                    pt = ps_big.tile([W, 4, P], FP, tag="pt", bufs=3)
                nc.tensor.transpose(pt[:, 2 * j2, :], xsb[:, b, 0:W], ident)
                nc.tensor.transpose(
                    pt[:, 2 * j2 + 1, :], xsb[:, b, W:BLK], ident
                )
                if j2 == 1:
                    xt4 = sb_t.tile([W, 4, P], FP, tag="xt4", bufs=8)
                    nc.vector.tensor_copy(xt4[:], pt[:])
                    xt8s.append(xt4)
        gmsb = const.tile([P, W], FP)
        nc.vector.tensor_copy(gmsb[0:C, :], gm_a[0:C, :])
        nc.scalar.activation(gmsb[C:P, :], gm_b[C:P, :], AF.Identity)

        # fold top+bottom into G [64,64] and m [64,1] (separate banks)
        f_g = ps_g.tile([C, C], FP, tag="chain")
        nc.tensor.matmul(f_g[:], e_top, gmsb[:, 1 : 1 + C], start=True, stop=False)
        nc.tensor.matmul(f_g[:], e_bot, gmsb[:, 0:C], start=False, stop=True)
        f_m = ps_g.tile([C, 1], FP, tag="chain")
        nc.tensor.matmul(f_m[:], e_top, gmsb[:, 0:1], start=True, stop=False)
        nc.tensor.matmul(f_m[:], e_bot, gmsb[:, C : C + 1], start=False, stop=True)

        # assemble Gt (65 x 65): [[G, m], [m^T, NROWS]]
        gt_sb = const.tile([W, W], FP)
        nc.vector.tensor_copy(gt_sb[0:C, 0:C], f_g[:])
        nc.scalar.activation(gt_sb[0:C, C:W], f_m[:], AF.Identity)
        mt_ps = ps_g.tile([1, C], FP, tag="chain")
        nc.tensor.transpose(mt_ps[:], gt_sb[0:C, C:W], id64)
        nc.vector.tensor_copy(gt_sb[C:W, 0:C], mt_ps[:])
        nc.vector.memset(gt_sb[C:W, C:W], float(N))

        # --- chain: T1 = Gt Wtv^T, then Mmat/c doubled on both halves -----
        # Host folds a Wq^T Wtk into uts and a Wtk^T bq into uqv, so
        # Mmat = Ww^T + uts^T T1 and c = T1^T uqv + bw.
        t1_ps = ps_g.tile([W, C], FP, tag="chain")
        nc.tensor.matmul(t1_ps[:], gt_sb[:], wvta)
        t1_sb = const.tile([W, C], FP)
        nc.vector.tensor_copy(t1_sb[:], t1_ps[:])
        # one matmul gives [Mmat-pre; c_row-pre]; one add applies Ww^T/bw.
        # The result IS Mt_odd = [Mmat; c_row]; Mt_even = [c_row; Mmat] is
        # a cyclic row shift done with one matmul against cyc.
        acr_ps = ps_g.tile([W, C], FP, tag="chain")
        nc.tensor.matmul(acr_ps[:], utq, t1_sb[:])
        m_od = const.tile([W, C], FP)
        nc.vector.tensor_add(m_od[:], acr_ps[:], wwbw)
        me_ps = ps_g.tile([W, C], FP, tag="chain")
        nc.tensor.matmul(me_ps[:], cyc, m_od[:])
        m_ev = const.tile([W, C], FP)
        nc.vector.tensor_copy(m_ev[:], me_ps[:])

        # --- own half: out = gelu(xt @ Mt) directly from PSUM ---------
        osb = const.tile([P, HPAIR, 2 * C], FP)
        orr = out_d[:].rearrange("(b p j) c -> p b (j c)", p=P, j=2)
        for g in range(8):  # 8 groups x 2 pairs (4 row-tiles)
            xt4 = xt8s[g]
            po = ps_big.tile([P, 4, C], FP, tag="po", bufs=3)
            # odd tiles first: m_od is ready two chain hops before m_ev
            for j in (1, 3, 0, 2):
                nc.tensor.matmul(
                    po[:, j, :], xt4[:, j, :],
                    m_ev[:] if j % 2 == 0 else m_od[:],
                )
            nc.scalar.activation(
                osb[:, 2 * g : 2 * g + 2, :].rearrange("p a c -> p (a c)"),
                po[:].rearrange("p a c -> p (a c)"),
                act_fn,
            )
            if g % 2 == 1:
                # one 256KB out-DMA per two groups: the SP ring issues
                # serially, and fewer/bigger transfers drain its queue
                # sooner at the tail
                nc.sync.dma_start(
                    out=orr[:, 2 * g - 2 : 2 * g + 2, :],
                    in_=osb[:, 2 * g - 2 : 2 * g + 2, :],
                )

    nc.compile()
    return nc


_NC_CACHE = None


def _get_nc() -> bass.Bass:
    global _NC_CACHE
    if _NC_CACHE is None:
        _NC_CACHE = build_nc()
    return _NC_CACHE


def make_wpk(inputs: dict) -> np.ndarray:
    Wq, Wk, Wv, Ww = (np.asarray(inputs[k], np.float32) for k in ("Wq", "Wk", "Wv", "Ww"))
    bq, bk, bv, bw = (np.asarray(inputs[k], np.float32) for k in ("bq", "bk", "bv", "bw"))
    wtk = np.concatenate([Wk, bk[:, None]], axis=1)          # [64, 65]
    um = (ALPHA * (Wq.T @ wtk)).astype(np.float32)           # [64, 65]
    uq = (ALPHA * (wtk.T @ bq)).astype(np.float32)           # [65]
    wpk = np.zeros((P, WPK_F), np.float32)
    wpk[0:C, WPK_VT : WPK_VT + C] = Wv.T
    wpk[C, WPK_VT : WPK_VT + C] = bv
    wpk[0:W, WPK_UTQ : WPK_UTQ + C] = um.T
    wpk[0:W, WPK_UTQ + C] = uq
    wpk[0:C, WPK_WB : WPK_WB + C] = Ww.T
    wpk[C, WPK_WB : WPK_WB + C] = bw
    wpk[np.arange(W), WPK_CYC + (np.arange(W) + 1) % W] = 1.0  # cyc
    wpk[:, WPK_ID : WPK_ID + P] = np.eye(P, dtype=np.float32)
    return wpk


def make_in_maps(inputs: dict) -> list[dict]:
    x = np.ascontiguousarray(np.asarray(inputs["x"], dtype=np.float32))
    wpk = np.ascontiguousarray(make_wpk(inputs))
    in_maps = []
    for c in range(NCORES):
        b, h = c // 2, c % 2
        if h == 0:
            xb = x[b]
        else:
            xb = np.concatenate([x[b, N // 2 :], x[b, : N // 2]], axis=0)
        arr = np.ones((P, NPAIR, BLK), np.float32)
        # row(p, pair, j) = pair*256 + 2p + j
        arr[:, :, 1 : 1 + 2 * C] = (
            xb.reshape(NPAIR, P, 2 * C).transpose(1, 0, 2)
        )
        in_maps.append(
            dict(xb=np.ascontiguousarray(arr.reshape(P, NPAIR * BLK)), wpk=wpk)
        )
    return in_maps


def kernel(**inputs) -> np.ndarray:
    nc = _get_nc()
    in_maps = make_in_maps(inputs)
    res = run_bass_kernel_spmd(nc, in_maps, list(range(NCORES)))
    out = np.empty((B, N, C), np.float32)
    for c in range(NCORES):
        b, h = c // 2, c % 2
        out[b, h * (N // 2) : (h + 1) * (N // 2)] = res.results[c]["out"]
    return out



# revision 3
# speedup vs baseline: 1.8973x; 1.8973x over previous
"""Trainium2 Bass kernel for nn_DenseGNOBlock (B=4, N=8192, C=64).

Reference computes, per batch b:
    q = x Wq^T + bq ; k = x Wk^T + bk ; v = x Wv^T + bv
    kernel = q k^T / sqrt(C) ; integral = kernel v / N
    out = gelu(x Ww^T + bw + integral)

No softmax, so the N x N kernel reassociates away completely. With the
ones-FIRST augmentation Xa = [1|x] (N x 65) and Wt* = [b*; W*^T] (65 x 64):
    Gt  = Xa^T Xa                         (65 x 65, symmetric)
    Mt  = Wtw + a Wtq Wtk^T Gt Wtv        (a = 1/(sqrt(C) N))
    out = gelu(Xa @ Mt)
Device chain: T1 = Gt Wtv ; acr = U T1 (U = a Wtq Wtk^T host-folded);
Mt = Wtw + acr. Everything else is layout engineering for the cost model:

- All matmul inputs are bf16 (PSUM accumulation stays fp32): 1 PE
  cycle/row instead of fp32's 4. End-to-end rel err ~3e-3.
- x ships twice, host-prepared in bf16: once as row-blocks for Gt
  (quad-packed [1|xa|1|xb|1|xc|1|xd] = 260 cols = 520B contiguous per
  partition, so no sub-512B DMA penalty), and once pre-TRANSPOSED
  (xt, own half only) for the output matmuls -- no on-chip transposes
  and no PSUM->SBUF transpose copies at all.
- The interleaved ones columns make each G matmul lhs/rhs [1|x_row]
  so the FULL Gt (including the row-sum border) accumulates into one
  PSUM tile over 64 matmuls: no fold, no m^T assembly, and both
  output halves use the same Mt (no cyclic-shift variant).
- DMAs spread over the three DMA-capable rings (SP, gpsimd/SWDGE,
  ACT after its activation-table load); input x lands ~0.6us earlier
  than a 2-ring split. Output is written bf16 (512B runs via the
  quad-interleaved row permutation baked into xt's column order) and
  upcast on the host.
- gelu reads PSUM directly in 4 ops of 512 elems: few enough that the
  fixed access latency does not dominate, big enough to pipeline with
  the final matmuls; each group's out-DMA issues immediately after.

Sharding: 8 cores, core c -> batch b = c//2, half h = c%2. Each core
receives the full x_b (for Gt) + its own transposed half, writes its half.
"""

import sys

for _p in ("/opt/trn_rl_repo", "/root/.axon_site/_ro/trn_rl_repo"):
    if _p not in sys.path:
        sys.path.append(_p)

import numpy as np
import ml_dtypes
from contextlib import ExitStack

import concourse.bass as bass
import concourse.bacc as bacc
import concourse.mybir as mybir
import concourse.tile as tile
from concourse.bass_utils import run_bass_kernel_spmd

FP = mybir.dt.float32
BF = mybir.dt.bfloat16
AF = mybir.ActivationFunctionType
BF_NP = ml_dtypes.bfloat16

B, N, C = 4, 8192, 64
P = 128               # partitions
W = C + 1             # augmented width (ones-first)
NBLK = 16             # quad row-blocks per batch (512 rows each)
BCOL = 4 * W          # 260 cols: [1|xa|1|xb|1|xc|1|xd]
NH = N // 2           # own half rows
NCH = NH // P         # 32 final-matmul chunks
NG2 = 4               # gelu groups (8 chunks = 1024 rows each)
NCORES = 8
ALPHA = 1.0 / (np.sqrt(np.float32(C)) * np.float32(N))
# packed weight layout (free offsets in wpk [65, 193])
WPK_TV = 0            # [0:65, 0:64]    Wtv = [bv; Wv^T]
WPK_UT = C            # [0:65, 64:129]  U^T, U = a Wtq Wtk^T
WPK_WB = C + W        # [0:65, 129:193] Wtw = [bw; Ww^T]
WPK_F = 2 * C + W     # 193
# x quad-block -> (engine, dma group) plan; G matmuls follow arrival order
XQ_SP = [(0, 3), (3, 6), (6, 8)]       # SP dma groups (block ranges)
XQ_PL = [(8, 11), (11, 14)]            # gpsimd dma groups
XQ_ACT = [(14, 16)]                    # ACT dma groups (after table load)
G_ORDER = [0, 1, 2, 3, 4, 5, 8, 9, 10, 6, 7, 11, 12, 13, 14, 15]


def build_nc(act: str = "gelu") -> bass.Bass:
    act_fn = {"gelu": AF.Gelu, "identity": AF.Identity}[act]
    nc = bacc.Bacc("TRN2", target_bir_lowering=False, debug=False)

    xq_d = nc.declare_dram_parameter("xq", [P, NBLK * BCOL], BF, isOutput=False)
    xt_d = nc.declare_dram_parameter("xt", [W, NH], BF, isOutput=False)
    wpk_d = nc.declare_dram_parameter("wpk", [W, WPK_F], BF, isOutput=False)
    out_d = nc.declare_dram_parameter("out", [NH, C], BF, isOutput=True)

    with ExitStack() as ctx:
        tc = ctx.enter_context(tile.TileContext(nc))
        const = ctx.enter_context(tc.tile_pool(name="const", bufs=1))
        ps_g = ctx.enter_context(tc.tile_pool(name="ps_g", bufs=2, space="PSUM"))
        ps_o = ctx.enter_context(tc.tile_pool(name="ps_o", bufs=4, space="PSUM"))

        wpk = const.tile([W, WPK_F], BF)
        wtv = wpk[:, WPK_TV : WPK_TV + C]
        ut = wpk[:, WPK_UT : WPK_UT + W]
        wwb = wpk[:, WPK_WB : WPK_WB + C]
        xqs = const.tile([P, NBLK, BCOL], BF)
        xts = const.tile([W, NH], BF)

        # --- input DMAs: three rings issue in parallel ------------------
        xqr = xq_d[:].rearrange("p (b k) -> p b k", k=BCOL)
        for a, b in XQ_SP:
            nc.sync.dma_start(out=xqs[:, a:b, :], in_=xqr[:, a:b, :])
        nc.gpsimd.dma_start(out=wpk[:], in_=wpk_d[:])
        for a, b in XQ_PL:
            nc.gpsimd.dma_start(out=xqs[:, a:b, :], in_=xqr[:, a:b, :])
        for a, b in XQ_ACT:
            nc.scalar.dma_start(out=xqs[:, a:b, :], in_=xqr[:, a:b, :])
        # xt needed only by the final matmuls; trails x on both rings
        nc.sync.dma_start(out=xts[:, 0 : NH // 2], in_=xt_d[:, 0 : NH // 2])
        nc.gpsimd.dma_start(out=xts[:, NH // 2 :], in_=xt_d[:, NH // 2 :])

        # --- Gt accumulation: one PSUM tile, 64 matmuls -----------------
        gt_ps = ps_g.tile([W, W], FP, tag="chain")
        nmm = NBLK * 4
        i = 0
        for blk in G_ORDER:
            for j in range(4):
                col = xqs[:, blk, j * W : (j + 1) * W]
                nc.tensor.matmul(
                    gt_ps[:], col, col, start=(i == 0), stop=(i == nmm - 1)
                )
                i += 1

        # --- chain: T1 = Gt Wtv ; Mt = Wtw + U T1 -----------------------
        gts = const.tile([W, W], BF)
        nc.vector.tensor_copy(gts[:], gt_ps[:])
        t1_ps = ps_g.tile([W, C], FP, tag="chain")
        nc.tensor.matmul(t1_ps[:], gts[:], wtv)   # Gt symmetric: gts^T = Gt
        t1s = const.tile([W, C], BF)
        nc.vector.tensor_copy(t1s[:], t1_ps[:])
        acr_ps = ps_g.tile([W, C], FP, tag="chain")
        nc.tensor.matmul(acr_ps[:], ut, t1s[:])
        mts = const.tile([W, C], BF)
        nc.vector.tensor_add(mts[:], acr_ps[:], wwb)

        # --- own half: out = gelu(Xa Mt), gelu straight from PSUM -------
        osb = const.tile([P, NG2, 2, 4, C], BF)
        orr = out_d[:].rearrange("(g u p j) c -> p g u j c", p=P, u=2, j=4)
        for g2 in range(NG2):
            po = ps_o.tile([P, 8, C], FP, tag="po", bufs=4)
            for k in range(8):
                ch = 8 * g2 + k
                nc.tensor.matmul(
                    po[:, k, :], xts[:, ch * P : (ch + 1) * P], mts[:]
                )
            nc.scalar.activation(
                osb[:, g2].rearrange("p u j c -> p (u j c)"),
                po[:].rearrange("p a c -> p (a c)"),
                act_fn,
            )
            # issue each group's out-DMA as soon as gelu lands; alternate
            # rings so the tail drains fast
            eng = nc.sync if g2 % 2 == 0 else nc.gpsimd
            eng.dma_start(out=orr[:, g2], in_=osb[:, g2])

    nc.compile()
    return nc


_NC_CACHE = None


def _get_nc() -> bass.Bass:
    global _NC_CACHE
    if _NC_CACHE is None:
        _NC_CACHE = build_nc()
    return _NC_CACHE


def make_wpk(inputs: dict) -> np.ndarray:
    Wq, Wk, Wv, Ww = (np.asarray(inputs[k], np.float32) for k in ("Wq", "Wk", "Wv", "Ww"))
    bq, bk, bv, bw = (np.asarray(inputs[k], np.float32) for k in ("bq", "bk", "bv", "bw"))
    wtq = np.concatenate([bq[None, :], Wq.T], axis=0)   # [65, 64]
    wtk = np.concatenate([bk[None, :], Wk.T], axis=0)
    wtv = np.concatenate([bv[None, :], Wv.T], axis=0)
    wwb = np.concatenate([bw[None, :], Ww.T], axis=0)
    u = (ALPHA * (wtq @ wtk.T)).astype(np.float32)      # [65, 65]
    wpk = np.zeros((W, WPK_F), np.float32)
    wpk[:, WPK_TV : WPK_TV + C] = wtv
    wpk[:, WPK_UT : WPK_UT + W] = u.T
    wpk[:, WPK_WB : WPK_WB + C] = wwb
    return wpk.astype(BF_NP)


def make_in_maps(inputs: dict) -> list[dict]:
    x = np.asarray(inputs["x"], dtype=np.float32).astype(BF_NP)  # [B, N, C]
    wpk = np.ascontiguousarray(make_wpk(inputs))
    in_maps = []
    for core in range(NCORES):
        b, h = core // 2, core % 2
        xb = x[b]
        # xq row-blocks: row(blk, p, j) = 512 blk + 4 p + j
        xq = np.ones((P, NBLK, 4, W), BF_NP)
        xq[:, :, :, 1:] = xb.reshape(NBLK, P, 4, C).transpose(1, 0, 2, 3)
        # xt own half, transposed, quad-interleaved column order:
        # column 128 c + p  <->  own row 512 (c//4) + 4 p + (c%4)
        xo = xb[h * NH : (h + 1) * NH]                   # [4096, 64]
        xt = np.ones((W, NH), BF_NP)
        xt[1:] = (
            xo.reshape(NH // 512, P, 4, C)               # [g, p, j, c]
            .transpose(3, 0, 2, 1)                       # [c, g, j, p]
            .reshape(C, NH)
        )
        in_maps.append(
            dict(
                xq=np.ascontiguousarray(xq.reshape(P, NBLK * BCOL)),
                xt=np.ascontiguousarray(xt),
                wpk=wpk,
            )
        )
    return in_maps


def kernel(**inputs) -> np.ndarray:
    nc = _get_nc()
    in_maps = make_in_maps(inputs)
    res = run_bass_kernel_spmd(nc, in_maps, list(range(NCORES)))
    out = np.empty((B, N, C), np.float32)
    for core in range(NCORES):
        b, h = core // 2, core % 2
        out[b, h * NH : (h + 1) * NH] = np.asarray(
            res.results[core]["out"], dtype=np.float32
        )
    return out


# revision 12
# speedup vs baseline: 2.2401x; 1.1807x over previous
"""Trainium2 Bass kernel for nn_DenseGNOBlock (B=4, N=8192, C=64).

Reference computes, per batch b:
    q = x Wq^T + bq ; k = x Wk^T + bk ; v = x Wv^T + bv
    kernel = q k^T / sqrt(C) ; integral = kernel v / N
    out = gelu(x Ww^T + bw + integral)

No softmax, so the N x N kernel reassociates away completely. With the
ones-FIRST augmentation Xa = [1|x] (N x 65) and Wt* = [b*; W*^T] (65 x 64):
    Gt  = Xa^T Xa                         (65 x 65, symmetric)
    Mt  = Wtw + a Wtq Wtk^T Gt Wtv        (a = 1/(sqrt(C) N))
    out = gelu(Xa @ Mt)
The rest is precision + layout engineering against the cost model:

- Gt's body accumulates from an fp8(e4m3) copy of x with DoubleRow
  matmuls: each instruction contracts TWO 128-row groups at 0.5 PE
  cycles/row, so the 8192-row Gram fits in 32 matmuls. The dual-fp8
  ldweights ISA check demands <=128 weight columns at an even,
  16B-aligned pair stride, so x ships in 80-col padded groups
  [1|x(64)|zeros(15)]; the ones column rides only the rhs (moving
  side), making each matmul yield [m | G] = Gt rows 1:65 into one
  PSUM tile. Gt's quantization noise averages out over N=8192: end-to-
  end rel err ~3e-3 (tolerance 2e-2).
- Gt's missing top row never gets materialized: the chain computes
  T1 = Gt Wtv in permuted row order (body first) from [m | G] alone --
  G Wv^T via symmetry, the m bv^T term through diag(m) (a per-partition
  tensor_scalar of the identity; no transposes anywhere), and T1's
  own-row via two 1-row matmuls; U's columns are host-permuted to
  match.
- Everything else is bf16 (PSUM accumulation stays fp32): 1 PE
  cycle/row instead of fp32's 4.
- x also ships pre-TRANSPOSED in bf16 (xt, own half only) for the
  output matmuls. Its column order bakes in the quad-interleaved row
  permutation that makes the bf16 output DMA 512B-contiguous.
- DMAs spread over the three DMA-capable rings (SP, gpsimd/SWDGE, and
  ACT for the weight pack behind its activation-table load). gelu
  reads PSUM in 3 ops sized [8,16,8] chunks: big enough that the fixed
  access latency doesn't dominate, staged so ACT never idles between
  the first matmul group and the last out-DMA (which ACT itself
  issues, keeping the slow SWDGE ring out of the drain tail).

Sharding: 8 cores, core c -> batch b = c//2, half h = c%2. Each core
receives the full x_b (for Gt) + its own transposed half, writes its half.
"""

import sys

for _p in ("/opt/trn_rl_repo", "/root/.axon_site/_ro/trn_rl_repo"):
    if _p not in sys.path:
        sys.path.append(_p)

import numpy as np
import ml_dtypes
from contextlib import ExitStack

import concourse.bass as bass
import concourse.bacc as bacc
import concourse.mybir as mybir
import concourse.tile as tile
from concourse.bass_utils import run_bass_kernel_spmd

FP = mybir.dt.float32
BF = mybir.dt.bfloat16
F8 = mybir.dt.float8e4
AF = mybir.ActivationFunctionType
DR = mybir.MatmulPerfMode.DoubleRow
BF_NP = ml_dtypes.bfloat16
F8_NP = ml_dtypes.float8_e4m3

B, N, C = 4, 8192, 64
P = 128               # partitions
W = C + 1             # augmented width (ones-first)
GW = 80               # padded group width: [1|x(64)|0(15)], 16B-aligned
NBLK = 8              # oct row-blocks per batch (1024 rows each)
BCOL = 8 * GW         # 640 cols per block
NH = N // 2           # own half rows
NCH = NH // P         # 32 final-matmul chunks
NCORES = 8
ALPHA = 1.0 / (np.sqrt(np.float32(C)) * np.float32(N))
# packed weight layout (free offsets in wpk [65, WPK_F])
WPK_UT = 0            # [0:65, 0:65]     (U~)^T, U~ = U cols rotated [1..64,0]
WPK_WB = W            # [0:65, 65:129]   Wtw = [bw; Ww^T]
WPK_WV = W + C        # [0:64, 129:193]  Wv^T
WPK_BV = W + 2 * C    # [0:64, 193:257]  ones x bv^T
WPK_ID = W + 3 * C    # [0:64, 257:321]  I64
WPK_NB = W + 4 * C    # [0:1, 321:385]   N * bv^T ; [0:1, 385] = 1.0
WPK_F = W + 5 * C + 1  # 386
G_ORDER = [0, 4, 5, 6, 7, 1, 2, 3]   # follows DMA arrival order
OGRP = [(0, 8), (8, 24), (24, 32)]   # gelu groups (chunk ranges, 4-aligned)


def build_nc(act: str = "gelu") -> bass.Bass:
    act_fn = {"gelu": AF.Gelu, "identity": AF.Identity}[act]
    nc = bacc.Bacc("TRN2", target_bir_lowering=False, debug=False)

    xq_d = nc.declare_dram_parameter("xq", [P, NBLK * BCOL], F8, isOutput=False)
    xt_d = nc.declare_dram_parameter("xt", [W, NH], BF, isOutput=False)
    wpk_d = nc.declare_dram_parameter("wpk", [W, WPK_F], BF, isOutput=False)
    out_d = nc.declare_dram_parameter("out", [NH, C], BF, isOutput=True)

    with ExitStack() as ctx:
        tc = ctx.enter_context(tile.TileContext(nc))
        const = ctx.enter_context(tc.tile_pool(name="const", bufs=1))
        ps_g = ctx.enter_context(tc.tile_pool(name="ps_g", bufs=2, space="PSUM"))
        ps_o = ctx.enter_context(tc.tile_pool(name="ps_o", bufs=1, space="PSUM"))

        wpk = const.tile([W, WPK_F], BF)
        ut = wpk[:, WPK_UT : WPK_UT + W]
        wwb = wpk[:, WPK_WB : WPK_WB + C]
        wvs = wpk[0:C, WPK_WV : WPK_WV + C]
        bvb = wpk[0:C, WPK_BV : WPK_BV + C]
        id64 = wpk[0:C, WPK_ID : WPK_ID + C]
        nbv = wpk[0:1, WPK_NB : WPK_NB + C]
        one1 = wpk[0:1, WPK_NB + C : WPK_NB + C + 1]
        xqs = const.tile([P, NBLK, 8, GW], F8)
        xts = const.tile([W, NH], BF)

        # --- input DMAs: three rings issue in parallel ------------------
        # first block ships alone so the Gram matmuls start at the DMA
        # round-trip floor; xt trails x on both rings (needed ~1.5us later)
        xqr = xq_d[:].rearrange("p (b k w) -> p b k w", k=8, w=GW)
        nc.sync.dma_start(out=xqs[:, 0:1], in_=xqr[:, 0:1])
        nc.sync.dma_start(out=xqs[:, 1:4], in_=xqr[:, 1:4])
        nc.gpsimd.dma_start(out=xqs[:, 4:6], in_=xqr[:, 4:6])
        nc.gpsimd.dma_start(out=xqs[:, 6:8], in_=xqr[:, 6:8])
        nc.scalar.dma_start(out=wpk[:], in_=wpk_d[:])
        nc.sync.dma_start(out=xts[:, 0 : NH // 2], in_=xt_d[:, 0 : NH // 2])
        nc.gpsimd.dma_start(out=xts[:, NH // 2 :], in_=xt_d[:, NH // 2 :])

        # --- Gt body accumulation: one PSUM tile, 32 DoubleRow matmuls --
        # lhsT = two x groups (128 weight cols, stride 80 = 16B-aligned);
        # rhs keeps the ones cols -> out accumulates [m | G] (Gt rows 1:65)
        gt_ps = ps_g.tile([C, W], FP, tag="chain")
        nmm = NBLK * 4
        i = 0
        for blk in G_ORDER:
            for j in range(4):
                pair = xqs[:, blk, 2 * j : 2 * j + 2]
                nc.tensor.matmul(
                    gt_ps[:], pair[:, :, 1 : 1 + C], pair[:, :, 0:W],
                    perf_mode=DR, start=(i == 0), stop=(i == nmm - 1),
                )
                i += 1

        # --- chain: T1 = Gt Wtv (row-permuted, body first) ; Mt --------
        gts = const.tile([C, W], BF)
        # ACT is idle here and Identity shares Gelu's table (gelu_and_others):
        # both chain copies ride ACT in parallel with the DVE diag(m) path
        nc.scalar.activation(gts[:], gt_ps[:], AF.Identity)
        msb = gts[:, 0:1]
        m32 = const.tile([C, 1], FP)      # fp32 twin: tensor_scalar wants it
        nc.vector.tensor_copy(m32[:], gt_ps[:, 0:1])
        diagm = const.tile([C, C], BF)
        nc.vector.tensor_scalar_mul(diagm[:], id64, m32[:])
        # T1 body = G Wv^T + m bv^T (G via symmetry, m bv^T via diag(m))
        t1b_ps = ps_g.tile([C, C], FP, tag="chain")
        nc.tensor.matmul(t1b_ps[:], gts[:, 1:W], wvs, start=True, stop=False)
        nc.tensor.matmul(t1b_ps[:], diagm[:], bvb, start=False, stop=True)
        # T1 top row = m^T Wv^T + N bv^T  (lands at permuted position 64)
        t1r_ps = ps_g.tile([1, C], FP, tag="r0")
        nc.tensor.matmul(t1r_ps[:], msb, wvs, start=True, stop=False)
        nc.tensor.matmul(t1r_ps[:], one1, nbv, start=False, stop=True)
        t1s = const.tile([W, C], BF)
        nc.vector.tensor_copy(t1s[0:C, :], t1b_ps[:])
        nc.scalar.activation(t1s[C : C + 1, :], t1r_ps[:], AF.Identity)
        acr_ps = ps_g.tile([W, C], FP, tag="chain")
        nc.tensor.matmul(acr_ps[:], ut, t1s[:])
        mts = const.tile([W, C], BF)
        nc.vector.tensor_add(mts[:], acr_ps[:], wwb)

        # --- own half: out = gelu(Xa Mt), gelu straight from PSUM -------
        # chunk c covers own rows 512*(c//4) + 4p + (c%4) (baked into xt's
        # column order), so a 4-aligned chunk group [c0,c1) maps to the
        # contiguous HBM row range [128*c0, 128*c1) with 512B runs
        osb = const.tile([P, NCH // 4, 4, C], BF)
        for gi, (c0, c1) in enumerate(OGRP):
            ng = c1 - c0
            po = ps_o.tile([P, ng, C], FP, tag=f"po{gi}")
            for k in range(ng):
                ch = c0 + k
                nc.tensor.matmul(
                    po[:, k, :], xts[:, ch * P : (ch + 1) * P], mts[:]
                )
            og = osb[:, c0 // 4 : c1 // 4]
            nc.scalar.activation(
                og.rearrange("p g j c -> p (g j c)"),
                po[:].rearrange("p a c -> p (a c)"),
                act_fn,
            )
            orr = out_d[128 * c0 : 128 * c1].rearrange(
                "(g p j) c -> p g j c", p=P, j=4
            )
            # last group's DMA issues from ACT right behind its own gelu;
            # earlier groups ride SP (the slower SWDGE ring would add its
            # bigger completion latency to the drain tail)
            eng = (nc.sync, nc.sync, nc.scalar)[gi]
            eng.dma_start(out=orr, in_=og)

    nc.compile()
    return nc


_NC_CACHE = None


def _get_nc() -> bass.Bass:
    global _NC_CACHE
    if _NC_CACHE is None:
        _NC_CACHE = build_nc()
    return _NC_CACHE


def make_wpk(inputs: dict) -> np.ndarray:
    Wq, Wk, Wv, Ww = (np.asarray(inputs[k], np.float32) for k in ("Wq", "Wk", "Wv", "Ww"))
    bq, bk, bv, bw = (np.asarray(inputs[k], np.float32) for k in ("bq", "bk", "bv", "bw"))
    wtq = np.concatenate([bq[None, :], Wq.T], axis=0)   # [65, 64]
    wtk = np.concatenate([bk[None, :], Wk.T], axis=0)
    wwb = np.concatenate([bw[None, :], Ww.T], axis=0)
    u = (ALPHA * (wtq @ wtk.T)).astype(np.float32)      # [65, 65]
    uperm = u[:, list(range(1, W)) + [0]]               # cols body-first
    wpk = np.zeros((W, WPK_F), np.float32)
    wpk[:, WPK_UT : WPK_UT + W] = uperm.T
    wpk[:, WPK_WB : WPK_WB + C] = wwb
    wpk[0:C, WPK_WV : WPK_WV + C] = Wv.T
    wpk[0:C, WPK_BV : WPK_BV + C] = bv[None, :]
    wpk[0:C, WPK_ID : WPK_ID + C] = np.eye(C, dtype=np.float32)
    wpk[0, WPK_NB : WPK_NB + C] = np.float32(N) * bv
    wpk[0, WPK_NB + C] = 1.0
    return wpk.astype(BF_NP)


def make_in_maps(inputs: dict) -> list[dict]:
    x = np.asarray(inputs["x"], dtype=np.float32)        # [B, N, C]
    x16 = x.astype(BF_NP)
    x8 = x.astype(F8_NP)
    wpk = np.ascontiguousarray(make_wpk(inputs))
    in_maps = []
    for core in range(NCORES):
        b, h = core // 2, core % 2
        # xq oct-blocks (fp8): row(blk, p, k) = 1024 blk + 8 p + k;
        # group layout [1 | x | 0*15] keeps the DoubleRow pair stride
        # 16B-aligned and the DMA runs 640B-contiguous
        xq = np.zeros((P, NBLK, 8, GW), F8_NP)
        xq[:, :, :, 0] = 1.0
        xq[:, :, :, 1 : 1 + C] = x8[b].reshape(NBLK, P, 8, C).transpose(1, 0, 2, 3)
        # xt own half (bf16), transposed, quad-interleaved column order:
        # column 128 c + p  <->  own row 512 (c//4) + 4 p + (c%4)
        xo = x16[b, h * NH : (h + 1) * NH]               # [4096, 64]
        xt = np.ones((W, NH), BF_NP)
        xt[1:] = (
            xo.reshape(NH // 512, P, 4, C)               # [g, p, j, c]
            .transpose(3, 0, 2, 1)                       # [c, g, j, p]
            .reshape(C, NH)
        )
        in_maps.append(
            dict(
                xq=np.ascontiguousarray(xq.reshape(P, NBLK * BCOL)),
                xt=np.ascontiguousarray(xt),
                wpk=wpk,
            )
        )
    return in_maps


def kernel(**inputs) -> np.ndarray:
    nc = _get_nc()
    in_maps = make_in_maps(inputs)
    res = run_bass_kernel_spmd(nc, in_maps, list(range(NCORES)))
    out = np.empty((B, N, C), np.float32)
    for core in range(NCORES):
        b, h = core // 2, core % 2
        out[b, h * NH : (h + 1) * NH] = np.asarray(
            res.results[core]["out"], dtype=np.float32
        )
    return out
